# revision 68
# baseline (speedup 1.0000x reference)
"""GAT layer (nn_GAT_Layer) as a Trainium2 Bass kernel, SPMD over 8 NeuronCores.

Math
----
With E[h,i,j] = e_l[h,i] + e_r[h,j] and A in {0,1}:
  exp(E) = exp(e_l) * exp(e_r)
  denom[h,i] = sum_j exp(E*A) = exp(e_l[h,i]) * (A @ exp(e_r[h]))[i] + (N - deg[i])
  out[h,i,:] = elu( (exp_el/denom)[h,i] * (A @ (exp_er[:,h,None] * HW[:,h,:]))[i] )
where HW = H @ W (per head), deg = A @ 1.

So the only O(N^2) work is one matmul  S = B^T @ A_rows^T  with
B = [G(64) | exp_er(8) | ones(1)]  -> [4096, 73]; everything else is tiny.

Sharding: rows of A are split across the 8 cores (512 rows each). Each core
redundantly computes B (cheap) and its own 512-row epilogue. No collectives.

Host passes A row-blocks pre-transposed so the contraction dim (j) lands on
SBUF partitions, plus a few constant 0/1 selection matrices (pure layout).
"""

import sys

if "/opt/trn_rl_repo" not in sys.path:
    sys.path.insert(0, "/opt/trn_rl_repo")

from contextlib import ExitStack

import numpy as np

import concourse.bass as bass
import concourse.tile as tile
from concourse import bacc, mybir
from concourse.bass_utils import run_bass_kernel_spmd

N, F, HEADS, U = 4096, 128, 8, 8
NCORES = 8
R = N // NCORES            # 512 rows per core
C = HEADS * U              # 64
NB = C + HEADS + 1         # 73 live columns of B
NB2 = 128                  # padded B width: [exp_er(8) | 1 | junk(9:64) | G(64:128)]
HR = R // 2                # 256: the epilogue processes i in two halves
JC = N // 128              # 32 contraction chunks
F32 = mybir.dt.float32
F32R = mybir.dt.float32r

# Big-matmul mode. Fields: a (A dtype), b (B dtype), terms (1 = single,
# 2 = hi+lo residual split of B), build (dtype of H^T / W operands of the
# B-build matmuls), el (dtype of the e_l matmul operands).
# A casts are exact (A is 0/1 so bf16/fp16/fp8e4 represent it exactly).
BF16, F16, F8 = mybir.dt.bfloat16, mybir.dt.float16, mybir.dt.float8e4
MODES = {
    "f32r":    dict(a=F32R, b=F32R, terms=1, build=F32R, el=F32R),
    "bf16x2":  dict(a=BF16, b=BF16, terms=2, build=F32R, el=F32R),
    "f16":     dict(a=F16, b=F16, terms=1, build=F32R, el=F32R),
    "f16a8":   dict(a=F8, b=F16, terms=1, build=F32R, el=F32R),
    "f16f":    dict(a=F16, b=F16, terms=1, build=F16, el=F16),
    "f16fa8":  dict(a=F8, b=F16, terms=1, build=F16, el=F16),
    "bf16x2f": dict(a=BF16, b=BF16, terms=2, build=F16, el=F16),
}
MODE = "f16a8"


def build_bass(reps=1, mode=None):
    """reps>1 repeats the whole body inside one NEFF (for delta timing)."""
    mode = mode or MODE
    cfg = MODES[mode]
    a_dt, b_dt, n_terms = cfg["a"], cfg["b"], cfg["terms"]
    build_dt, el_dt = cfg["build"], cfg["el"]

    nc = bacc.Bacc("TRN2", target_bir_lowering=False, debug=True)

    # per-core inputs
    at = nc.declare_dram_parameter("at", [N, R], a_dt, isOutput=False)  # A[rows,:].T
    # one packed fp32 constants input: [wt | alrd | repc | degc | hrt | wfc]
    # cols 0:128 wt(rows 0:64), 128:144 alrd(rows 0:64), 144:208 repc(rows
    # 0:8), 208:216 degc(rows 0:9), 216:728 hrt, 728:792 wfc
    cc = nc.declare_dram_parameter("cc", [128, 792], F32, isOutput=False)
    ht = nc.declare_dram_parameter("ht", [F, N], F32 if build_dt == F32R else build_dt, isOutput=False)  # H.T
    # output (transposed): o[h*U+u, i_local]
    o = nc.declare_dram_parameter("o", [C, R], F32, isOutput=True)

    AF = mybir.ActivationFunctionType
    OP = mybir.AluOpType

    with tile.TileContext(nc) as tc, ExitStack() as ctx:
        consts = ctx.enter_context(tc.tile_pool(name="consts", bufs=2))
        bigp = ctx.enter_context(tc.tile_pool(name="bigp", bufs=2))
        apool = ctx.enter_context(tc.tile_pool(name="apool", bufs=4))
        epool = ctx.enter_context(tc.tile_pool(name="epool", bufs=2))
        bps = ctx.enter_context(tc.tile_pool(name="bps", bufs=4, space="PSUM"))
        spool = ctx.enter_context(tc.tile_pool(name="spool", bufs=1, space="PSUM"))
        mpsum = ctx.enter_context(tc.tile_pool(name="mpsum", bufs=2, space="PSUM"))

        def emit_body():
            # ---- constant / shared loads ----
            # split the 2 MiB H^T load across 8 DMA queues so it doesn't
            # serialize behind one queue (it gates every B-build matmul)
            if build_dt == F32R:
                ht_f32 = bigp.tile([F, N], F32, tag="ht_f32")
                for q in range(4):
                    nc.sync.dma_start(
                        out=ht_f32[:, q * (N // 4) : (q + 1) * (N // 4)],
                        in_=ht[:, q * (N // 4) : (q + 1) * (N // 4)])
                ht_sb = bigp.tile([F, N], build_dt, tag="ht_sb")
                for q in range(4):
                    nc.vector.tensor_copy(
                        out=ht_sb[:, q * (N // 4) : (q + 1) * (N // 4)],
                        in_=ht_f32[:, q * (N // 4) : (q + 1) * (N // 4)])
            else:
                ht_sb = bigp.tile([F, N], build_dt, tag="ht_sb")
                for q in range(4):
                    nc.sync.dma_start(
                        out=ht_sb[:, q * (N // 4) : (q + 1) * (N // 4)],
                        in_=ht[:, q * (N // 4) : (q + 1) * (N // 4)])
            cc_sb = consts.tile([128, 792], F32, tag="cc_sb")
            nc.sync.dma_start(out=cc_sb, in_=cc[:, :])
            wt_sb = cc_sb[0:C, 0:F]
            alrd_sb = cc_sb[0:C, 128:144]
            hrt_sb = consts.tile([F, R], el_dt, tag="hrt_sb")
            nc.vector.tensor_copy(out=hrt_sb, in_=cc_sb[:, 216:728])
            # rhs_ext cols: 0:8 -> WR (e_r), 8:64 -> 0, 64:128 -> W (HW).
            # S rows come out as [Se 0:8 | deg 8 | junk 9:64 | Sg 64:128];
            # the junk rows are never read - M=128 padding is free since
            # matmul cost is stream-bound, and it puts Se/deg at base
            # partition 0 (f32r-legal tile_position) and Sg at base 64.
            RW = 256 if build_dt == F32R else NB2
            rhs_ext = consts.tile([F, RW], build_dt, tag="rhs_ext")
            nc.vector.tensor_copy(out=rhs_ext[:, C:NB2], in_=cc_sb[:, 728:792])
            if build_dt == F32R:
                # f32r memset is ISA-illegal; stage zeros via a DVE copy
                zz = consts.tile([F, 1], F32, tag="zz")
                nc.vector.memset(zz, 0.0)
                nc.vector.tensor_copy(out=rhs_ext[:, 8:C],
                                      in_=zz.to_broadcast((F, C - 8)))
                nc.vector.tensor_copy(out=rhs_ext[:, NB2:RW],
                                      in_=zz.to_broadcast((F, RW - NB2)))
            else:
                nc.vector.memset(rhs_ext[:, 8:C], 0.0)
            degc_sb = consts.tile([128, 8], F32R, tag="degc_sb")
            nc.vector.tensor_copy(out=degc_sb[0:9, :], in_=cc_sb[0:9, 208:216])
            repc_sb = cc_sb[0:8, 144:208]

            # ---- WL | WR : [f, 16] = wt.T @ alrd ----
            wlr_ps = mpsum.tile([128, R], F32, tag="mp")
            nc.tensor.matmul(wlr_ps[:, 0:16], lhsT=wt_sb[:, :], rhs=alrd_sb[:, :],
                             start=True, stop=True)
            wl_sb = consts.tile([F, 8], el_dt, tag="wl_sb")
            nc.vector.tensor_copy(out=wl_sb, in_=wlr_ps[:, 0:8])
            nc.vector.tensor_copy(out=rhs_ext[:, 0:8], in_=wlr_ps[:, 8:16])

            # ---- e_l for this core's rows: el[h, i] at partitions 0:8 ----
            el_ps = mpsum.tile([128, R], F32, tag="mp")
            for h in range(2):
                hs = slice(h * HR, (h + 1) * HR)
                nc.tensor.matmul(el_ps[0:8, hs], lhsT=wl_sb[:, :],
                                 rhs=hrt_sb[:, hs], start=True, stop=True)
            expel_t = epool.tile([128, R], F32, tag="expel")
            expel = expel_t[0:8, :]
            nc.scalar.activation(out=expel, in_=el_ps[0:8, :], func=AF.Exp)

            # ---- B chunks: b_all[:, t, c, :] = [exp_er|1|0|G] terms ----
            b_all = bigp.tile([F, n_terms, JC, NB2], b_dt, tag="b_all")
            for t in range(n_terms):
                nc.gpsimd.memset(b_all[:, t, :, 9:C], 0.0)
            for c in range(JC):
                pb = bps.tile([128, RW], F32, tag="pb")
                nc.tensor.matmul(pb[:, :], lhsT=ht_sb[:, c * 128 : (c + 1) * 128],
                                 rhs=rhs_ext[:, :], start=True, stop=True)
                if n_terms == 1:
                    # write exp and the product straight into B (short chain)
                    nc.scalar.activation(out=b_all[:, 0, c, 0:9], in_=pb[:, 0:9],
                                         func=AF.Exp)
                    nc.vector.tensor_tensor(
                        b_all[:, 0, c, C:NB2].rearrange("p (h u) -> p h u", u=U),
                        pb[:, C:NB2].rearrange("p (h u) -> p h u", u=U),
                        b_all[:, 0, c, 0:HEADS][:, :, None]
                        .to_broadcast((F, HEADS, U)),
                        OP.mult,
                    )
                else:
                    # g_sb = fp32 [exp_er|1|0|G], then hi + residual lo terms
                    g_sb = apool.tile([F, NB2], F32, tag="g_sb")
                    nc.scalar.activation(out=g_sb[:, 0:9], in_=pb[:, 0:9],
                                         func=AF.Exp)
                    nc.vector.tensor_tensor(
                        g_sb[:, C:NB2].rearrange("p (h u) -> p h u", u=U),
                        pb[:, C:NB2].rearrange("p (h u) -> p h u", u=U),
                        g_sb[:, 0:HEADS][:, :, None].to_broadcast((F, HEADS, U)),
                        OP.mult,
                    )
                    nc.vector.tensor_copy(out=b_all[:, 0, c, 0:9],
                                          in_=g_sb[:, 0:9])
                    nc.vector.tensor_copy(out=b_all[:, 0, c, C:NB2],
                                          in_=g_sb[:, C:NB2])
                    nc.vector.tensor_sub(out=b_all[:, 1, c, 0:9],
                                         in0=g_sb[:, 0:9],
                                         in1=b_all[:, 0, c, 0:9])
                    nc.vector.tensor_sub(out=b_all[:, 1, c, C:NB2],
                                         in0=g_sb[:, C:NB2],
                                         in1=b_all[:, 0, c, C:NB2])

            # ---- main matmul, split into two i-halves so each half's
            # epilogue chain overlaps the other half's work ----
            s_h = [spool.tile([128, HR], F32, tag=f"s{h}", name=f"s{h}")
                   for h in range(2)]
            at_r = at.rearrange("(cc p) i -> p cc i", p=128)  # [128, 32, 512]
            GRP = 8
            a_tiles = []
            for g in range(JC // GRP):
                a_sb = apool.tile([128, GRP, R], a_dt, tag="a", name=f"a{g}")
                nc.sync.dma_start(out=a_sb, in_=at_r[:, g * GRP : (g + 1) * GRP, :])
                a_tiles.append(a_sb)
            # all half-0 matmuls first: half 0 finishes at the midpoint so its
            # epilogue chain overlaps half 1's matmuls
            for h in range(2):
                for g in range(JC // GRP):
                    for k in range(GRP):
                        c = g * GRP + k
                        for t in range(n_terms):
                            nc.tensor.matmul(
                                s_h[h][:, :], lhsT=b_all[:, t, c, :],
                                rhs=a_tiles[g][:, k, h * HR : (h + 1) * HR],
                                start=(c == 0 and t == 0),
                                stop=(c == JC - 1 and t == n_terms - 1))

            # ---- epilogue per half ----
            sed_t = epool.tile([128, R], F32R, tag="sed")
            den_t = epool.tile([128, R], F32, tag="den")
            rec_t = epool.tile([128, R], F32, tag="rec")
            ratio_t = epool.tile([128, R], F32, tag="ratio")
            dgc_ps = mpsum.tile([128, R], F32, tag="mp")
            rep_ps = mpsum.tile([128, R], F32, tag="mp")
            sg_sb = epool.tile([128, R], F32, tag="sg")
            pre = epool.tile([128, R], F32, tag="pre")
            relu_t = epool.tile([128, R], F32, tag="relu_t")
            mint = epool.tile([128, R], F32, tag="mint")
            expm = epool.tile([128, R], F32, tag="expm")
            out_sb = epool.tile([128, R], F32, tag="out_sb")
            for h in range(2):
                hs = slice(h * HR, (h + 1) * HR)
                s_ps = s_h[h]
                # -deg via a tiny f32r matmul at base partition 0
                nc.vector.tensor_copy(out=sed_t[0:9, hs], in_=s_ps[0:9, :])
                nc.tensor.matmul(dgc_ps[0:8, hs], lhsT=degc_sb[0:9, :],
                                 rhs=sed_t[0:9, hs], start=True, stop=True)
                # denom = exp_el * Se + 4096 - deg;  ratio = exp_el / denom
                nc.vector.tensor_tensor(den_t[0:8, hs], s_ps[0:8, :],
                                        expel_t[0:8, hs], OP.mult)
                nc.vector.scalar_tensor_tensor(den_t[0:8, hs], den_t[0:8, hs],
                                               float(N), dgc_ps[0:8, hs],
                                               OP.add, OP.add)
                nc.vector.reciprocal(out=rec_t[0:8, hs], in_=den_t[0:8, hs])
                nc.vector.tensor_mul(out=ratio_t[0:8, hs], in0=expel_t[0:8, hs],
                                     in1=rec_t[0:8, hs])
                # replicate ratio[h] over units -> partitions 64:128
                nc.tensor.matmul(rep_ps[C:NB2, hs], lhsT=repc_sb[0:8, :],
                                 rhs=ratio_t[0:8, hs], start=True, stop=True)
                nc.scalar.activation(out=sg_sb[C:NB2, hs], in_=s_ps[C:NB2, :],
                                     func=AF.Copy)
                nc.vector.tensor_mul(out=pre[C:NB2, hs], in0=rep_ps[C:NB2, hs],
                                     in1=sg_sb[C:NB2, hs])
                # elu(x) = relu(x) + exp(min(x, 0)) - 1
                nc.scalar.activation(out=relu_t[C:NB2, hs], in_=pre[C:NB2, hs],
                                     func=AF.Relu)
                nc.scalar.activation(out=mint[C:NB2, hs], in_=pre[C:NB2, hs],
                                     func=AF.Relu, scale=-1.0)   # -min(x,0)
                nc.scalar.activation(out=expm[C:NB2, hs], in_=mint[C:NB2, hs],
                                     func=AF.Exp, scale=-1.0)    # exp(min(x,0))
                nc.vector.scalar_tensor_tensor(out_sb[C:NB2, hs],
                                               relu_t[C:NB2, hs], -1.0,
                                               expm[C:NB2, hs], OP.add, OP.add)
            nc.sync.dma_start(out=o[:, :], in_=out_sb[C:NB2, :])

        for _ in range(reps):
            emit_body()

    nc.compile()
    return nc


def host_inputs(A, H, W, a_left, a_right, mode=None):
    """Shard + relayout the full inputs into per-core in_maps (no arithmetic;
    the A cast to bf16/fp16 is exact since A is 0/1)."""
    mode = mode or MODE
    cfg = MODES[mode]
    at_np = mybir.dt.np(cfg["a"])
    build_np = mybir.dt.np(cfg["build"])
    el_np = mybir.dt.np(cfg["el"])
    A = np.ascontiguousarray(np.asarray(A, dtype=np.float32))
    H = np.ascontiguousarray(np.asarray(H, dtype=np.float32))
    W = np.asarray(W, dtype=np.float32)
    a_left = np.asarray(a_left, dtype=np.float32)
    a_right = np.asarray(a_right, dtype=np.float32)

    ht = np.ascontiguousarray(H.T).astype(build_np)                   # [128, 4096]
    HT32 = np.ascontiguousarray(H.T)                                  # fp32 view
    # packed constants [128, 792]: wt | alrd | repc | degc | hrt | wfc
    base = np.zeros((128, 792), np.float32)
    base[0:C, 0:F] = W.transpose(0, 2, 1).reshape(C, F)               # wt
    for h in range(HEADS):                                            # alrd
        base[h * U : (h + 1) * U, 128 + h] = a_left[h]
        base[h * U : (h + 1) * U, 136 + h] = a_right[h]
        base[h, 144 + h * U : 144 + (h + 1) * U] = 1.0                # repc
    base[8, 208:216] = -1.0                                           # degc row 8
    base[:, 728:792] = W.transpose(1, 0, 2).reshape(F, C)             # wfc

    in_maps = []
    for k in range(NCORES):
        rows = slice(k * R, (k + 1) * R)
        cc = base.copy()
        cc[:, 216:728] = HT32[:, rows]                                # hrt
        in_maps.append(dict(
            ht=ht,
            cc=cc,
            at=np.ascontiguousarray(A[rows, :].T).astype(at_np),
        ))
    return in_maps


_NC_CACHE = {}


def _get_nc(reps=1, mode=None):
    key = (reps, mode or MODE)
    if key not in _NC_CACHE:
        _NC_CACHE[key] = build_bass(reps, mode)
    return _NC_CACHE[key]


def run(A, H, W, a_left, a_right, trace=False, **spmd_kwargs):
    nc = _get_nc()
    in_maps = host_inputs(A, H, W, a_left, a_right)
    res = run_bass_kernel_spmd(nc, in_maps, core_ids=list(range(NCORES)),
                               trace=trace, **spmd_kwargs)
    out = np.concatenate([res.results[k]["o"].T for k in range(NCORES)], axis=0)
    return np.ascontiguousarray(out, dtype=np.float32), res


def kernel(A, H, W, a_left, a_right):
    out, _ = run(A, H, W, a_left, a_right, trace=False)
    return out


# revision 69
# speedup vs baseline: 1.0988x; 1.0988x over previous
"""GAT layer (nn_GAT_Layer) as a Trainium2 Bass kernel, SPMD over 8 NeuronCores.

Math
----
With E[h,i,j] = e_l[h,i] + e_r[h,j] and A in {0,1}:
  exp(E) = exp(e_l) * exp(e_r)
  denom[h,i] = sum_j exp(E*A) = exp(e_l[h,i]) * (A @ exp(e_r[h]))[i] + (N - deg[i])
  out[h,i,:] = elu( (exp_el/denom)[h,i] * (A @ (exp_er[:,h,None] * HW[:,h,:]))[i] )
where HW = H @ W (per head), deg = A @ 1.

So the only O(N^2) work is one matmul  S = B^T @ A_rows^T  with
B = [G(64) | exp_er(8) | ones(1)]  -> [4096, 73]; everything else is tiny.

Sharding: rows of A are split across the 8 cores (512 rows each). Each core
redundantly computes B (cheap) and its own 512-row epilogue. No collectives.

Host passes A row-blocks pre-transposed so the contraction dim (j) lands on
SBUF partitions, plus a few constant 0/1 selection matrices (pure layout).
"""

import sys

if "/opt/trn_rl_repo" not in sys.path:
    sys.path.insert(0, "/opt/trn_rl_repo")

from contextlib import ExitStack

import numpy as np

import concourse.bass as bass
import concourse.tile as tile
from concourse import bacc, mybir
from concourse.bass_utils import run_bass_kernel_spmd

N, F, HEADS, U = 4096, 128, 8, 8
NCORES = 8
R = N // NCORES            # 512 rows per core
C = HEADS * U              # 64
NB = C + HEADS + 1         # 73 live columns of B
NB2 = 128                  # padded B width: [exp_er(8) | 1 | junk(9:64) | G(64:128)]
HR = R // 2                # 256: the epilogue processes i in two halves
JC = N // 128              # 32 contraction chunks
F32 = mybir.dt.float32
F32R = mybir.dt.float32r

# Big-matmul mode. Fields: a (A dtype), b (B dtype), terms (1 = single,
# 2 = hi+lo residual split of B), build (dtype of H^T / W operands of the
# B-build matmuls), el (dtype of the e_l matmul operands).
# A casts are exact (A is 0/1 so bf16/fp16/fp8e4 represent it exactly).
BF16, F16, F8 = mybir.dt.bfloat16, mybir.dt.float16, mybir.dt.float8e4
MODES = {
    "f32r":    dict(a=F32R, b=F32R, terms=1, build=F32R, el=F32R),
    "bf16x2":  dict(a=BF16, b=BF16, terms=2, build=F32R, el=F32R),
    "f16":     dict(a=F16, b=F16, terms=1, build=F32R, el=F32R),
    "f16a8":   dict(a=F8, b=F16, terms=1, build=F32R, el=F32R),
    "f16f":    dict(a=F16, b=F16, terms=1, build=F16, el=F16),
    "f16fa8":  dict(a=F8, b=F16, terms=1, build=F16, el=F16),
    "bf16x2f": dict(a=BF16, b=BF16, terms=2, build=F16, el=F16),
}
MODE = "f16a8"


def build_bass(reps=1, mode=None):
    """reps>1 repeats the whole body inside one NEFF (for delta timing)."""
    mode = mode or MODE
    cfg = MODES[mode]
    a_dt, b_dt, n_terms = cfg["a"], cfg["b"], cfg["terms"]
    build_dt, el_dt = cfg["build"], cfg["el"]

    nc = bacc.Bacc("TRN2", target_bir_lowering=False, debug=True)

    # per-core inputs
    at = nc.declare_dram_parameter("at", [N, R], a_dt, isOutput=False)  # A[rows,:].T
    # one packed fp32 constants input: [wt | alrd | repc | degc | hrt | wfc]
    # cols 0:128 wt(rows 0:64), 128:144 alrd(rows 0:64), 144:208 repc(rows
    # 0:8), 208:216 degc(rows 0:9), 216:728 hrt, 728:792 wfc
    cc = nc.declare_dram_parameter("cc", [128, 792], F32, isOutput=False)
    ht = nc.declare_dram_parameter("ht", [F, N], F32 if build_dt == F32R else build_dt, isOutput=False)  # H.T
    # output (transposed): o[h*U+u, i_local]
    o = nc.declare_dram_parameter("o", [C, R], F32, isOutput=True)

    AF = mybir.ActivationFunctionType
    OP = mybir.AluOpType

    with tile.TileContext(nc) as tc, ExitStack() as ctx:
        consts = ctx.enter_context(tc.tile_pool(name="consts", bufs=2))
        bigp = ctx.enter_context(tc.tile_pool(name="bigp", bufs=2))
        apool = ctx.enter_context(tc.tile_pool(name="apool", bufs=4))
        epool = ctx.enter_context(tc.tile_pool(name="epool", bufs=2))
        bps = ctx.enter_context(tc.tile_pool(name="bps", bufs=4, space="PSUM"))
        spool = ctx.enter_context(tc.tile_pool(name="spool", bufs=1, space="PSUM"))
        mpsum = ctx.enter_context(tc.tile_pool(name="mpsum", bufs=2, space="PSUM"))

        def emit_body():
            # ---- constant / shared loads ----
            # split the 2 MiB H^T load across 8 DMA queues so it doesn't
            # serialize behind one queue (it gates every B-build matmul)
            if build_dt == F32R:
                ht_f32 = bigp.tile([F, N], F32, tag="ht_f32")
                for q in range(4):
                    nc.sync.dma_start(
                        out=ht_f32[:, q * (N // 4) : (q + 1) * (N // 4)],
                        in_=ht[:, q * (N // 4) : (q + 1) * (N // 4)])
                ht_sb = bigp.tile([F, N], build_dt, tag="ht_sb")
                for q in range(4):
                    eng = nc.vector if q % 2 == 0 else nc.scalar
                    if q % 2 == 0:
                        nc.vector.tensor_copy(
                            out=ht_sb[:, q * (N // 4) : (q + 1) * (N // 4)],
                            in_=ht_f32[:, q * (N // 4) : (q + 1) * (N // 4)])
                    else:
                        nc.scalar.activation(
                            out=ht_sb[:, q * (N // 4) : (q + 1) * (N // 4)],
                            in_=ht_f32[:, q * (N // 4) : (q + 1) * (N // 4)],
                            func=mybir.ActivationFunctionType.Copy)
            else:
                ht_sb = bigp.tile([F, N], build_dt, tag="ht_sb")
                for q in range(4):
                    nc.sync.dma_start(
                        out=ht_sb[:, q * (N // 4) : (q + 1) * (N // 4)],
                        in_=ht[:, q * (N // 4) : (q + 1) * (N // 4)])
            cc_sb = consts.tile([128, 792], F32, tag="cc_sb")
            nc.sync.dma_start(out=cc_sb, in_=cc[:, :])
            wt_sb = cc_sb[0:C, 0:F]
            alrd_sb = cc_sb[0:C, 128:144]
            hrt_sb = consts.tile([F, R], el_dt, tag="hrt_sb")
            nc.vector.tensor_copy(out=hrt_sb, in_=cc_sb[:, 216:728])
            # rhs_ext cols: 0:8 -> WR (e_r), 8:64 -> 0, 64:128 -> W (HW).
            # S rows come out as [Se 0:8 | deg 8 | junk 9:64 | Sg 64:128];
            # the junk rows are never read - M=128 padding is free since
            # matmul cost is stream-bound, and it puts Se/deg at base
            # partition 0 (f32r-legal tile_position) and Sg at base 64.
            RW = 256 if build_dt == F32R else NB2
            rhs_ext = consts.tile([F, RW], build_dt, tag="rhs_ext")
            nc.vector.tensor_copy(out=rhs_ext[:, C:NB2], in_=cc_sb[:, 728:792])
            if build_dt == F32R:
                # f32r memset is ISA-illegal; stage zeros via a DVE copy
                zz = consts.tile([F, 1], F32, tag="zz")
                nc.vector.memset(zz, 0.0)
                nc.vector.tensor_copy(out=rhs_ext[:, 8:C],
                                      in_=zz.to_broadcast((F, C - 8)))
                nc.vector.tensor_copy(out=rhs_ext[:, NB2:RW],
                                      in_=zz.to_broadcast((F, RW - NB2)))
            else:
                nc.vector.memset(rhs_ext[:, 8:C], 0.0)
            degc_sb = consts.tile([128, 8], F32R, tag="degc_sb")
            nc.vector.tensor_copy(out=degc_sb[0:9, :], in_=cc_sb[0:9, 208:216])
            repc_sb = cc_sb[0:8, 144:208]

            # ---- WL | WR : [f, 16] = wt.T @ alrd ----
            wlr_ps = mpsum.tile([128, R], F32, tag="mp")
            nc.tensor.matmul(wlr_ps[:, 0:16], lhsT=wt_sb[:, :], rhs=alrd_sb[:, :],
                             start=True, stop=True)
            wl_sb = consts.tile([F, 8], el_dt, tag="wl_sb")
            nc.vector.tensor_copy(out=wl_sb, in_=wlr_ps[:, 0:8])
            nc.vector.tensor_copy(out=rhs_ext[:, 0:8], in_=wlr_ps[:, 8:16])

            # ---- e_l for this core's rows: el[h, i] at partitions 0:8 ----
            el_ps = mpsum.tile([128, R], F32, tag="mp")
            for h in range(2):
                hs = slice(h * HR, (h + 1) * HR)
                nc.tensor.matmul(el_ps[0:8, hs], lhsT=wl_sb[:, :],
                                 rhs=hrt_sb[:, hs], start=True, stop=True)
            expel_t = epool.tile([128, R], F32, tag="expel")
            expel = expel_t[0:8, :]
            nc.scalar.activation(out=expel, in_=el_ps[0:8, :], func=AF.Exp)

            # ---- B chunks: b_all[:, t, c, :] = [exp_er|1|0|G] terms ----
            b_all = bigp.tile([F, n_terms, JC, NB2], b_dt, tag="b_all")
            for t in range(n_terms):
                nc.gpsimd.memset(b_all[:, t, :, 9:C], 0.0)
            for cp in range(JC // 2):
                pb = bps.tile([128, 2, RW], F32, tag="pb")
                for j in range(2):
                    c = 2 * cp + j
                    nc.tensor.matmul(pb[:, j, :],
                                     lhsT=ht_sb[:, c * 128 : (c + 1) * 128],
                                     rhs=rhs_ext[:, :], start=True, stop=True)
                if n_terms == 1:
                    # one exp for both chunks, then per-chunk products
                    nc.scalar.activation(
                        out=b_all[:, 0, 2 * cp : 2 * cp + 2, 0:9],
                        in_=pb[:, :, 0:9], func=AF.Exp)
                    for j in range(2):
                        c = 2 * cp + j
                        nc.vector.tensor_tensor(
                            b_all[:, 0, c, C:NB2].rearrange(
                                "p (h u) -> p h u", u=U),
                            pb[:, j, C:NB2].rearrange("p (h u) -> p h u", u=U),
                            b_all[:, 0, c, 0:HEADS][:, :, None]
                            .to_broadcast((F, HEADS, U)),
                            OP.mult,
                        )
                else:
                    for j in range(2):
                        c = 2 * cp + j
                        g_sb = apool.tile([F, NB2], F32, tag="g_sb")
                        nc.scalar.activation(out=g_sb[:, 0:9],
                                             in_=pb[:, j, 0:9], func=AF.Exp)
                        nc.vector.tensor_tensor(
                            g_sb[:, C:NB2].rearrange("p (h u) -> p h u", u=U),
                            pb[:, j, C:NB2].rearrange("p (h u) -> p h u", u=U),
                            g_sb[:, 0:HEADS][:, :, None]
                            .to_broadcast((F, HEADS, U)),
                            OP.mult,
                        )
                        nc.vector.tensor_copy(out=b_all[:, 0, c, 0:9],
                                              in_=g_sb[:, 0:9])
                        nc.vector.tensor_copy(out=b_all[:, 0, c, C:NB2],
                                              in_=g_sb[:, C:NB2])
                        nc.vector.tensor_sub(out=b_all[:, 1, c, 0:9],
                                             in0=g_sb[:, 0:9],
                                             in1=b_all[:, 0, c, 0:9])
                        nc.vector.tensor_sub(out=b_all[:, 1, c, C:NB2],
                                             in0=g_sb[:, C:NB2],
                                             in1=b_all[:, 0, c, C:NB2])

            # ---- main matmul, split into two i-halves so each half's
            # epilogue chain overlaps the other half's work ----
            s_h = [spool.tile([128, HR], F32, tag=f"s{h}", name=f"s{h}")
                   for h in range(2)]
            at_r = at.rearrange("(cc p) i -> p cc i", p=128)  # [128, 32, 512]
            GRP = 8
            a_tiles = []
            for g in range(JC // GRP):
                a_sb = apool.tile([128, GRP, R], a_dt, tag="a", name=f"a{g}")
                nc.sync.dma_start(out=a_sb, in_=at_r[:, g * GRP : (g + 1) * GRP, :])
                a_tiles.append(a_sb)
            # all half-0 matmuls first: half 0 finishes at the midpoint so its
            # epilogue chain overlaps half 1's matmuls
            for h in range(2):
                for g in range(JC // GRP):
                    for k in range(GRP):
                        c = g * GRP + k
                        for t in range(n_terms):
                            nc.tensor.matmul(
                                s_h[h][:, :], lhsT=b_all[:, t, c, :],
                                rhs=a_tiles[g][:, k, h * HR : (h + 1) * HR],
                                start=(c == 0 and t == 0),
                                stop=(c == JC - 1 and t == n_terms - 1))

            # ---- epilogue per half ----
            sed_t = epool.tile([128, R], F32R, tag="sed")
            den_t = epool.tile([128, R], F32, tag="den")
            rec_t = epool.tile([128, R], F32, tag="rec")
            ratio_t = epool.tile([128, R], F32, tag="ratio")
            dgc_ps = mpsum.tile([128, R], F32, tag="mp")
            rep_ps = mpsum.tile([128, R], F32, tag="mp")
            sg_sb = epool.tile([128, R], F32, tag="sg")
            pre = epool.tile([128, R], F32, tag="pre")
            relu_t = epool.tile([128, R], F32, tag="relu_t")
            mint = epool.tile([128, R], F32, tag="mint")
            expm = epool.tile([128, R], F32, tag="expm")
            out_sb = epool.tile([128, R], F32, tag="out_sb")
            for h in range(2):
                hs = slice(h * HR, (h + 1) * HR)
                s_ps = s_h[h]
                # -deg via a tiny f32r matmul at base partition 0
                nc.vector.tensor_copy(out=sed_t[0:9, hs], in_=s_ps[0:9, :])
                nc.tensor.matmul(dgc_ps[0:8, hs], lhsT=degc_sb[0:9, :],
                                 rhs=sed_t[0:9, hs], start=True, stop=True)
                # denom = exp_el * Se + 4096 - deg;  ratio = exp_el / denom
                nc.vector.tensor_tensor(den_t[0:8, hs], s_ps[0:8, :],
                                        expel_t[0:8, hs], OP.mult)
                nc.vector.scalar_tensor_tensor(den_t[0:8, hs], den_t[0:8, hs],
                                               float(N), dgc_ps[0:8, hs],
                                               OP.add, OP.add)
                nc.vector.reciprocal(out=rec_t[0:8, hs], in_=den_t[0:8, hs])
                nc.vector.tensor_mul(out=ratio_t[0:8, hs], in0=expel_t[0:8, hs],
                                     in1=rec_t[0:8, hs])
                # replicate ratio[h] over units -> partitions 64:128
                nc.tensor.matmul(rep_ps[C:NB2, hs], lhsT=repc_sb[0:8, :],
                                 rhs=ratio_t[0:8, hs], start=True, stop=True)
                nc.scalar.activation(out=sg_sb[C:NB2, hs], in_=s_ps[C:NB2, :],
                                     func=AF.Copy)
                nc.vector.tensor_mul(out=pre[C:NB2, hs], in0=rep_ps[C:NB2, hs],
                                     in1=sg_sb[C:NB2, hs])
                # elu(x) = relu(x) + exp(min(x, 0)) - 1
                nc.scalar.activation(out=relu_t[C:NB2, hs], in_=pre[C:NB2, hs],
                                     func=AF.Relu)
                nc.scalar.activation(out=mint[C:NB2, hs], in_=pre[C:NB2, hs],
                                     func=AF.Relu, scale=-1.0)   # -min(x,0)
                nc.scalar.activation(out=expm[C:NB2, hs], in_=mint[C:NB2, hs],
                                     func=AF.Exp, scale=-1.0)    # exp(min(x,0))
                nc.vector.scalar_tensor_tensor(out_sb[C:NB2, hs],
                                               relu_t[C:NB2, hs], -1.0,
                                               expm[C:NB2, hs], OP.add, OP.add)
            nc.sync.dma_start(out=o[:, :], in_=out_sb[C:NB2, :])

        for _ in range(reps):
            emit_body()

    nc.compile()
    return nc


def host_inputs(A, H, W, a_left, a_right, mode=None):
    """Shard + relayout the full inputs into per-core in_maps (no arithmetic;
    the A cast to bf16/fp16 is exact since A is 0/1)."""
    mode = mode or MODE
    cfg = MODES[mode]
    at_np = mybir.dt.np(cfg["a"])
    build_np = mybir.dt.np(cfg["build"])
    el_np = mybir.dt.np(cfg["el"])
    A = np.ascontiguousarray(np.asarray(A, dtype=np.float32))
    H = np.ascontiguousarray(np.asarray(H, dtype=np.float32))
    W = np.asarray(W, dtype=np.float32)
    a_left = np.asarray(a_left, dtype=np.float32)
    a_right = np.asarray(a_right, dtype=np.float32)

    ht = np.ascontiguousarray(H.T).astype(build_np)                   # [128, 4096]
    HT32 = np.ascontiguousarray(H.T)                                  # fp32 view
    # packed constants [128, 792]: wt | alrd | repc | degc | hrt | wfc
    base = np.zeros((128, 792), np.float32)
    base[0:C, 0:F] = W.transpose(0, 2, 1).reshape(C, F)               # wt
    for h in range(HEADS):                                            # alrd
        base[h * U : (h + 1) * U, 128 + h] = a_left[h]
        base[h * U : (h + 1) * U, 136 + h] = a_right[h]
        base[h, 144 + h * U : 144 + (h + 1) * U] = 1.0                # repc
    base[8, 208:216] = -1.0                                           # degc row 8
    base[:, 728:792] = W.transpose(1, 0, 2).reshape(F, C)             # wfc

    in_maps = []
    for k in range(NCORES):
        rows = slice(k * R, (k + 1) * R)
        cc = base.copy()
        cc[:, 216:728] = HT32[:, rows]                                # hrt
        in_maps.append(dict(
            ht=ht,
            cc=cc,
            at=np.ascontiguousarray(A[rows, :].T).astype(at_np),
        ))
    return in_maps


_NC_CACHE = {}


def _get_nc(reps=1, mode=None):
    key = (reps, mode or MODE)
    if key not in _NC_CACHE:
        _NC_CACHE[key] = build_bass(reps, mode)
    return _NC_CACHE[key]


def run(A, H, W, a_left, a_right, trace=False, **spmd_kwargs):
    nc = _get_nc()
    in_maps = host_inputs(A, H, W, a_left, a_right)
    res = run_bass_kernel_spmd(nc, in_maps, core_ids=list(range(NCORES)),
                               trace=trace, **spmd_kwargs)
    out = np.concatenate([res.results[k]["o"].T for k in range(NCORES)], axis=0)
    return np.ascontiguousarray(out, dtype=np.float32), res


def kernel(A, H, W, a_left, a_right):
    out, _ = run(A, H, W, a_left, a_right, trace=False)
    return out
